# revision 35
# baseline (speedup 1.0000x reference)
"""Trainium2 Bass kernel for nn_BatchedVQLAMDecoder (8-core SPMD).

Sharding: mod-8 interleave of the 4120-token sequence (8 chunks of 515,
padded to 520). Core k owns padded rows p == k (mod 8) of every chunk.
Block-causal mask at chunk granularity => uniform SPMD.

v5 (from v3):
- Cross-layer overlap: the next layer's LN1 / transposes / K,V / Q and
  the KV ship + AllGather run interleaved with the current layer's MLP
  w2 row-groups, so collectives start ~2 row-groups into the MLP.
- 2 AllGathers per layer (4 chunks each) instead of 8 small ones.
- Extras keys (the 8-wide key tail of every chunk) pulled out of the
  per-chunk inner loop into one masked per-layer pass: ~20% fewer QK/AV
  matmul passes and softmax exps.
- Software-pipelined attention inner loop: QK(i+1) issues before
  exp/AV(i) so PE keeps executing while the scalar engine runs exp.
- Transposes via PE (identity matmul) + scalar-engine copy instead of
  the serial DMA-transpose queue; QKV PSUM->SBUF moves with bias on the
  scalar engine (activation Identity); residual biases folded into the
  PSUM accumulation via a ones-row matmul.
- No rowmask multiply (padding handled by the extras mask).
"""

import threading

import numpy as np

import bass_rust
import concourse.bass as bass
import concourse.tile as tile
from concourse import bacc, mybir
from concourse.bass_utils import run_bass_kernel_spmd
from concourse.bass_interp import get_hw_module

F32 = mybir.dt.float32
F16 = mybir.dt.float16
U8 = mybir.dt.uint8
AF = mybir.ActivationFunctionType

T, N, D, CDim, E, H, DEPTH = 8, 256, 1024, 128, 512, 8, 3
Dh = E // H                   # 64
CHUNK = 2 * N + 3             # 515
JJ = 65                       # local rows per (core, chunk)
PC = 8 * JJ                   # padded chunk = 520
LR = T * JJ                   # local rows per core = 520
LRP = 528                     # padded to mult-16 for dma transpose
NC_ = 8
SCALE = 1.0 / np.sqrt(Dh)

KBYTES = E * JJ * 2           # fp16 K^T slab bytes per chunk = 66560
VW = 768                      # V slab width: 4x [V_even(64)|ones(64)|V_odd(64)]
VBYTES = JJ * VW * 2          # fp16 V slab bytes per chunk = 99840
CKVB = KBYTES + VBYTES        # 166400
PAIRB = 4 * CKVB              # AG payload: 4 chunks

RT_REAL = [128, 128, 128, 128, 8]


def _row_spans(lo, cnt):
    out = []
    done = 0
    while done < cnt:
        g = lo + done
        t = g // 128
        po = g % 128
        c = min(128 - po, cnt - done)
        out.append((t, po, c, done))
        done += c
    return out


def _mk_ap(x, dims):
    c = x.copy()
    c.ap = bass_rust.VecI64Pair([[int(a), int(b)] for a, b in dims])
    return c


def _pstride(x):
    return int([list(p) for p in x.ap][0][0])


def build_program():
    nc = bacc.Bacc("TRN2", target_bir_lowering=False, debug=False,
                   num_devices=NC_)

    def inp(name, shape, dt):
        return nc.dram_tensor(name, list(shape), dt, kind="ExternalInput").ap()

    eye_d = inp("eye", [128, 128], F16)
    zT_d = inp("zT", [D, 2 * CDim], F16)
    posz_d = inp("posz", [2 * CDim, E], F32)
    rest_d = inp("rest", [T * 33, E], F32)
    xmask_d = inp("xmask", [64, 2 * PC], F32)
    pw_d = inp("patch_w", [D, E], F16)
    pb_d = inp("patch_b", [E], F32)
    qkw_d = inp("qkw16", [DEPTH, 128, 4, 2 * E], F16)
    vw_d = inp("vw16", [DEPTH, 128, 4, E], F16)
    ow_d = inp("ow16", [DEPTH, 128, 4, E], F16)
    w1_d = inp("w1_16", [DEPTH, 128, 4, 4 * E], F16)
    w2_d = inp("w2_16", [DEPTH, 128, 16, E], F16)
    pb24_d = inp("pb24", [DEPTH, 128, 24], F32)
    lnv_d = inp("lnvec", [DEPTH, 7, E], F16)
    ng_d = inp("norm_g", [E], F32)
    nb_d = inp("norm_b", [E], F32)
    opw_d = inp("oproj_w", [E, D], F16)
    opb_d = inp("oproj_b", [D], F16)

    out_d = nc.dram_tensor("out", [T * 33, D], F32, kind="ExternalOutput").ap()

    kv_loc = [nc.dram_tensor(f"kv_loc{h}", [PAIRB], U8).ap() for h in range(2)]
    kv_g = [nc.dram_tensor(f"kv_g{h}", [NC_, PAIRB], U8,
                           addr_space="Shared").ap() for h in range(2)]

    with tile.TileContext(nc) as tc:
        import contextlib
        ctx = contextlib.ExitStack()
        with ctx:
            persist = ctx.enter_context(tc.tile_pool(name="persist", bufs=1))
            work = ctx.enter_context(tc.tile_pool(name="work", bufs=2))
            wts = ctx.enter_context(tc.tile_pool(name="wts", bufs=1))

            eps_t = persist.tile([128, 1], F32, tag="eps")
            nc.vector.memset(eps_t, 1e-5)
            xm_sb = persist.tile([64, 2, PC], F32, tag="xm")
            nc.sync.dma_start(
                out=xm_sb, in_=xmask_d.rearrange("p (h q) -> p h q", h=2))
            # V chunk tiles (2 parity): per head pair pr:
            # [V(2pr) 64 | ones 64 | V(2pr+1) 64]; 4 main key slabs only
            vc = [persist.tile([128, 4, VW], F16, tag=f"vc{par}",
                               name=f"vc{par}") for par in range(2)]
            for par in range(2):
                x = vc[par][:, 0, 64:128]
                ones_ap = _mk_ap(x, [[_pstride(x), 128], [VW, 4], [192, 4],
                                     [1, 64]])
                nc.vector.memset(ones_ap, 1.0)
            seq = persist.tile([128, 5, E], F32, tag="seq", name="seq")
            nc.vector.memset(seq, 0.0)
            eye_sb = persist.tile([128, 128], F16, tag="eye")
            nc.sync.dma_start(out=eye_sb, in_=eye_d)
            ones_row = persist.tile([1, LRP], F16, tag="ones_row")
            nc.vector.memset(ones_row, 1.0)

            def _rsqrt(rs, mv, cnt):
                nc.scalar.activation(out=rs[:cnt], in_=mv[:cnt, 1:2],
                                     func=AF.Sqrt, bias=eps_t[:cnt], scale=1.0)
                nc.vector.reciprocal(out=rs[:cnt], in_=rs[:cnt])

            # ------------- front end -------------
            with tc.tile_pool(name="front", bufs=1) as fp, \
                 tc.tile_pool(name="frontp", bufs=2, space="PSUM") as fpp:
                pb_bc = fp.tile([128, E], F32, tag="pb_bc")
                nc.sync.dma_start(out=pb_bc, in_=pb_d.partition_broadcast(128))
                zT_sb = fp.tile([128, 8, 256], F16, tag="zT")
                nc.sync.dma_start(
                    out=zT_sb, in_=zT_d.rearrange("(c p) n -> p c n", p=128))
                pw_sb = fp.tile([128, 8, E], F16, tag="pw")
                nc.sync.dma_start(
                    out=pw_sb, in_=pw_d.rearrange("(c p) n -> p c n", p=128))
                posz_sb = fp.tile([128, 2, E], F32, tag="posz")
                nc.sync.dma_start(
                    out=posz_sb, in_=posz_d.rearrange("(t p) e -> p t e", p=128))

                for zt in range(2):
                    ps = fpp.tile([128, E], F32, tag="psZ")
                    for cd in range(8):
                        nc.tensor.matmul(ps, zT_sb[:, cd, 128 * zt:128 * (zt + 1)],
                                         pw_sb[:, cd, :],
                                         start=(cd == 0), stop=(cd == 7))
                    zf = fp.tile([128, E], F32, tag="zf")
                    nc.vector.tensor_add(out=zf, in0=ps, in1=pb_bc)
                    stats = fp.tile([128, 6], F32, tag="zstats")
                    nc.vector.bn_stats(out=stats, in_=zf)
                    mv = fp.tile([128, 2], F32, tag="zmv")
                    nc.vector.bn_aggr(out=mv, in_=stats)
                    rs = fp.tile([128, 1], F32, tag="zrs")
                    _rsqrt(rs, mv, 128)
                    zn = fp.tile([128, E], F32, tag="zn")
                    nc.vector.tensor_scalar(out=zn, in0=zf,
                                            scalar1=mv[:, 0:1], scalar2=rs,
                                            op0=mybir.AluOpType.subtract,
                                            op1=mybir.AluOpType.mult)
                    nc.vector.tensor_add(out=zn, in0=zn, in1=posz_sb[:, zt, :])
                    for cb in range(4):
                        c = 4 * zt + cb
                        for (g, po, cnt, off) in _row_spans(JJ * c, 32):
                            nc.sync.dma_start(
                                out=seq[po:po + cnt, g, :],
                                in_=zn[32 * cb + off:32 * cb + off + cnt, :])
                for c in range(T):
                    for (g, po, cnt, off) in _row_spans(JJ * c + 32, 33):
                        nc.sync.dma_start(
                            out=seq[po:po + cnt, g, :],
                            in_=rest_d[33 * c + off:33 * c + off + cnt, :])

            # ================= layers =================
            SHIP_AFTER = {1: (0, None), 2: (2, 0), 3: (4, None), 4: (6, 1)}

            def load_weights(li):
                w = {}
                w["lnv"] = wts.tile([128, 7, E], F16, tag="lnv", bufs=2, name="lnv")
                nc.sync.dma_start(
                    out=w["lnv"],
                    in_=lnv_d[li].rearrange("v e -> (v e)")
                    .partition_broadcast(128).rearrange("p (v e) -> p v e", v=7))
                w["pb24"] = wts.tile([128, 24], F32, tag="pb24", bufs=2, name="pb24")
                nc.sync.dma_start(out=w["pb24"], in_=pb24_d[li])
                w["qkw"] = wts.tile([128, 4, 2 * E], F16, tag="qkw", name="qkw")
                nc.sync.dma_start(out=w["qkw"], in_=qkw_d[li])
                w["vw"] = wts.tile([128, 4, E], F16, tag="vw", name="vw")
                nc.sync.dma_start(out=w["vw"], in_=vw_d[li])
                w["ow"] = wts.tile([128, 4, E], F16, tag="ow", name="ow")
                nc.sync.dma_start(out=w["ow"], in_=ow_d[li])
                w["w1"] = wts.tile([128, 4, 4 * E], F16, tag="w1", name="w1")
                nc.sync.dma_start(out=w["w1"], in_=w1_d[li])
                w["w2"] = wts.tile([128, 16, E], F16, tag="w2", name="w2")
                nc.sync.dma_start(out=w["w2"], in_=w2_d[li])
                return w

            def _ship_pair(hs, c0, grp):
                for c in (c0, c0 + 1):
                    base = (c % 4) * CKVB
                    gw = c // 4
                    kdst = (kv_loc[gw][base:base + KBYTES]
                            .bitcast(F16)
                            .rearrange("(g p j) -> p g j", g=4, p=128))
                    nc.sync.dma_start(out=kdst,
                                      in_=hs["qkK"][:, :, JJ * c:JJ * c + JJ])
                    vdst = (kv_loc[gw][base + KBYTES:base + CKVB]
                            .bitcast(F16)
                            .rearrange("(l e) -> l e", e=VW))
                    for (g, po, cnt, off) in _row_spans(JJ * c, JJ):
                        nc.sync.dma_start(
                            out=vdst[off:off + cnt, :],
                            in_=hs["v16"][po:po + cnt, g, :])
                if grp is not None:
                    nc.gpsimd.collective_compute(
                        "AllGather", mybir.AluOpType.bypass,
                        replica_groups=[list(range(NC_))],
                        ins=[kv_loc[grp][:]], outs=[kv_g[grp][:, :]])

            def _ln_group(dst, g, gi, bi, lnv_sb):
                cnt = 16 if g == 4 else 128
                veng = nc.vector if g % 2 == 0 else nc.gpsimd
                stats = work.tile([128, 6], F32, tag="ln_st")
                nc.vector.bn_stats(out=stats[:cnt], in_=seq[:cnt, g, :])
                mv = work.tile([128, 2], F32, tag="ln_mv")
                nc.vector.bn_aggr(out=mv[:cnt], in_=stats[:cnt])
                rs = work.tile([128, 1], F32, tag="ln_rs")
                _rsqrt(rs, mv, cnt)
                y = work.tile([128, E], F32, tag="ln_y")
                nc.vector.tensor_scalar(out=y[:cnt], in0=seq[:cnt, g, :],
                                        scalar1=mv[:cnt, 0:1],
                                        scalar2=rs[:cnt],
                                        op0=mybir.AluOpType.subtract,
                                        op1=mybir.AluOpType.mult)
                veng.tensor_mul(out=y[:cnt], in0=y[:cnt],
                                in1=lnv_sb[:cnt, gi, :])
                veng.tensor_add(out=dst[:cnt, g, :], in0=y[:cnt],
                                in1=lnv_sb[:cnt, bi, :])

            def head_step(w, hs, hp, g):
                """LN1 group g of the next layer, its transposes (via PE),
                V(g), then the chunk-pair K/Q + ship + AllGather."""
                if g == 0:
                    hs["h16"] = work.tile([128, 5, E], F16, tag="h1",
                                          name="h1n", bufs=1)
                    hs["hT"] = work.tile([128, 4, LRP], F16, tag="hT1",
                                         name="hT1", bufs=1)
                    hs["v16"] = work.tile([128, 5, VW], F16, tag="v16",
                                          name="v16", bufs=1)
                    hs["qkQ"] = work.tile([128, 4, LRP], F16, tag="qkQ",
                                          name="qkQ", bufs=1)
                    hs["qkK"] = work.tile([128, 4, LRP], F16, tag="qkK",
                                          name="qkK", bufs=1)
                _ln_group(hs["h16"], g, 0, 1, w["lnv"])
                pcnt = 16 if g == 4 else 128
                for e in range(4):
                    ptr = hp.tile([128, 128], F16, tag="ptr")
                    nc.tensor.transpose(
                        ptr[:, :pcnt],
                        hs["h16"][0:pcnt, g, 128 * e:128 * (e + 1)],
                        eye_sb[:pcnt, :pcnt])
                    nc.scalar.activation(
                        out=hs["hT"][:, e, 128 * g:128 * g + pcnt],
                        in_=ptr[:, :pcnt], func=AF.Copy)
                real = RT_REAL[g]
                ps = hp.tile([128, 512], F32, tag="ph")
                for e in range(4):
                    nc.tensor.matmul(ps[:real],
                                     hs["hT"][:, e, 128 * g:128 * g + real],
                                     w["vw"][:, e, :],
                                     start=(e == 0), stop=(e == 3))

                def _s4(x, st, real=real):
                    return _mk_ap(x, [[_pstride(x), real], [st, 4], [1, 64]])
                v16 = hs["v16"]
                nc.vector.memset(_s4(v16[:real, g, 64:128], 192), 1.0)
                for par in range(2):
                    nc.vector.tensor_add(
                        out=_s4(v16[:real, g, 128 * par:128 * par + 64], 192),
                        in0=_s4(ps[:real, 64 * par:64 * par + 64], 128),
                        in1=_s4(w["lnv"][:real, 4, 64 * par:64 * par + 64],
                                128))
                if g in SHIP_AFTER:
                    c0, grp = SHIP_AFTER[g]
                    # K^T and Q^T columns for this chunk pair
                    for m in range(8):
                        ps2 = hp.tile([128, 512], F32, tag="ph")
                        for e in range(4):
                            nc.tensor.matmul(
                                ps2[:, :2 * JJ],
                                w["qkw"][:, e, 128 * m:128 * (m + 1)],
                                hs["hT"][:, e, JJ * c0:JJ * c0 + 2 * JJ],
                                start=(e == 0), stop=(e == 3))
                        dst = (hs["qkQ"][:, m, JJ * c0:JJ * c0 + 2 * JJ]
                               if m < 4 else
                               hs["qkK"][:, m - 4, JJ * c0:JJ * c0 + 2 * JJ])
                        nc.scalar.activation(
                            out=dst, in_=ps2[:, :2 * JJ], func=AF.Identity,
                            bias=w["pb24"][:, m:m + 1])
                    _ship_pair(hs, c0, grp)

            # head of layer 0 (after front end)
            w_cur = load_weights(0)
            hs_cur = {}
            with tc.tile_pool(name="hp0", bufs=2, space="PSUM") as hp0:
                for g in range(5):
                    head_step(w_cur, hs_cur, hp0, g)

            for li in range(DEPTH):
                w_next = load_weights(li + 1) if li + 1 < DEPTH else None
                lnv_sb = w_cur["lnv"]
                pb24_sb = w_cur["pb24"]
                ow_sb = w_cur["ow"]
                w1_sb = w_cur["w1"]
                w2_sb = w_cur["w2"]
                qkQ = hs_cur["qkQ"]
                hs_next = {}

                # ---- attention (software-pipelined) ----
                oT = [work.tile([128, 2, PC], F32, tag=f"oT{pr}",
                                name=f"oT{pr}", bufs=1) for pr in range(4)]
                ktx = work.tile([128, 4, 64], F16, tag="ktx", bufs=1)
                vx = [work.tile([32, VW], F16, tag=f"vx{i}",
                                name=f"vx{i}", bufs=1) for i in range(2)]
                with tc.tile_pool(name="pS", bufs=2, space="PSUM") as pS, \
                     tc.tile_pool(name="pO", bufs=2, space="PSUM") as pO:
                    pend = []

                    def run_step(st):
                        (kind, pr, s, psq, poT, rlo, rcnt, cpr) = st
                        eq = work.tile([128, 2, PC], F16, tag="eq", bufs=3)
                        if kind == "main":
                            nc.scalar.activation(
                                out=eq[:, :, :rcnt], in_=psq[:, :, :rcnt],
                                func=AF.Exp, scale=float(SCALE))
                            vcp = vc[cpr % 2]
                            for hh in range(2):
                                off = 192 * pr + 64 * hh
                                nc.tensor.matmul(
                                    poT[:, hh, :rcnt],
                                    vcp[:, s, off:off + 128],
                                    eq[:, hh, :rcnt],
                                    start=(s == 0), stop=(s == 3))
                        else:
                            (k0, kn) = s
                            nc.scalar.activation(
                                out=eq[:kn, :, :rcnt], in_=psq[:kn, :, :rcnt],
                                func=AF.Exp, scale=float(SCALE))
                            for hh in range(2):
                                off = 192 * pr + 64 * hh
                                nc.tensor.matmul(
                                    poT[:, hh, :rcnt],
                                    vx[k0 // 32][:, off:off + 128],
                                    eq[:kn, hh, :rcnt],
                                    start=True, stop=True)
                        if (kind == "main" and s == 3) or kind == "x":
                            if kind == "main" and cpr == 0:
                                nc.vector.tensor_copy(
                                    out=oT[pr][:, :, rlo:rlo + rcnt],
                                    in_=poT[:, :, :rcnt])
                            else:
                                nc.vector.tensor_add(
                                    out=oT[pr][:, :, rlo:rlo + rcnt],
                                    in0=oT[pr][:, :, rlo:rlo + rcnt],
                                    in1=poT[:, :, :rcnt])

                    def push(st):
                        pend.append(st)
                        if len(pend) > 1:
                            run_step(pend.pop(0))

                    for cpr in range(T):
                        grp, rel = cpr // 4, cpr % 4
                        base = rel * CKVB
                        ktc = work.tile([128, 4, PC], F16, tag="ktc", bufs=2)
                        ksrc = (kv_g[grp][:, base:base + KBYTES]
                                .bitcast(F16)
                                .rearrange("r (g p j) -> p g r j", g=4, p=128))
                        for qb in range(4):
                            nc.sync.dma_start(
                                out=ktc[:, qb, 0:512],
                                in_=ksrc[:, qb:qb + 1, :, 0:64].squeeze(1))
                            nc.sync.dma_start(
                                out=ktc[:, qb, 512:520],
                                in_=ksrc[:, qb:qb + 1, :, 64:65]
                                .squeeze(3).squeeze(1))
                        vcp = vc[cpr % 2]
                        vsrc = (kv_g[grp][:, base + KBYTES:base + CKVB]
                                .bitcast(F16)
                                .rearrange("r (l e) -> r l e", e=VW))
                        # col kappa = 64r + l (l<64)
                        for par0 in range(2):
                            dst0 = vcp[64 * par0:64 * par0 + 64, 0, :]
                            d3 = _mk_ap(dst0, [[_pstride(dst0), 64],
                                               [VW, 4], [1, VW]])
                            nc.sync.dma_start(
                                out=d3,
                                in_=vsrc[par0:8:2, 0:64, :]
                                .rearrange("r l e -> l r e"))
                        # extras keys (l=64) of this chunk -> persistent slabs
                        nc.gpsimd.tensor_copy(
                            out=ktx[:, :, 8 * cpr:8 * cpr + 8],
                            in_=ktc[:, :, 512:520])
                        vxd = vx[cpr // 4][8 * (cpr % 4):8 * (cpr % 4) + 8, :]
                        nc.sync.dma_start(out=vxd,
                                          in_=vsrc[:, 64:65, :].squeeze(1))

                        passes = ([(0, 512), (512, 8)] if cpr == 0
                                  else [(JJ * cpr, LR - JJ * cpr)])
                        for (rlo, rcnt) in passes:
                            for pr in range(4):
                                poT = pO.tile([128, 2, 512], F32, tag="poT")
                                for s in range(4):
                                    psq = pS.tile([128, 2, 512], F32,
                                                  tag="psq")
                                    lo = 128 * s
                                    for hh in range(2):
                                        nc.tensor.matmul(
                                            psq[:, hh, :rcnt],
                                            ktc[64 * hh:64 * hh + 64,
                                                pr, lo:lo + 128],
                                            qkQ[64 * hh:64 * hh + 64,
                                                pr, rlo:rlo + rcnt],
                                            start=True, stop=True)
                                    push(("main", pr, s, psq, poT,
                                          rlo, rcnt, cpr))
                        if cpr in (3, 7):
                            # extras keys of chunks cpr-3..cpr, masked
                            k0 = 8 * (cpr - 3)
                            for (rlo, rcnt) in ((0, 512), (512, 8)):
                                for pr in range(4):
                                    poT = pO.tile([128, 2, 512], F32,
                                                  tag="poT")
                                    psq = pS.tile([128, 2, 512], F32,
                                                  tag="psq")
                                    for hh in range(2):
                                        nc.tensor.matmul(
                                            psq[:32, hh, :rcnt],
                                            ktx[64 * hh:64 * hh + 64,
                                                pr, k0:k0 + 32],
                                            qkQ[64 * hh:64 * hh + 64,
                                                pr, rlo:rlo + rcnt],
                                            start=True, stop=True)
                                    nc.vector.tensor_add(
                                        out=psq[:32, :, :rcnt],
                                        in0=psq[:32, :, :rcnt],
                                        in1=xm_sb[k0:k0 + 32, :,
                                                  rlo:rlo + rcnt])
                                    push(("x", pr, (k0, 32), psq, poT,
                                          rlo, rcnt, None))

                    while pend:
                        run_step(pend.pop(0))

                # ---- normalize (denominators already in oT) ----
                # slot0: o(2pr) @p0..64, den(2pr) @p64..128
                # slot1: den(2pr+1) @p0..64, o(2pr+1) @p64..128
                oTn = work.tile([128, 4, LRP], F16, tag="oTn", name="oTn",
                                bufs=1)
                for pr in range(4):
                    eng = nc.vector if pr % 2 == 0 else nc.gpsimd
                    dn = work.tile([128, 2, PC], F32, tag="dn", bufs=1)
                    nc.sync.dma_start(out=dn[0:64, 0, :],
                                      in_=oT[pr][64:128, 0, :])
                    nc.sync.dma_start(out=dn[64:128, 1, :],
                                      in_=oT[pr][0:64, 1, :])
                    nc.vector.reciprocal(out=dn[0:64, 0, :],
                                         in_=dn[0:64, 0, :])
                    nc.vector.reciprocal(out=dn[64:128, 1, :],
                                         in_=dn[64:128, 1, :])
                    eng.tensor_mul(out=oTn[0:64, pr, 0:PC],
                                   in0=oT[pr][0:64, 0, :],
                                   in1=dn[0:64, 0, :])
                    eng.tensor_mul(out=oTn[64:128, pr, 0:PC],
                                   in0=oT[pr][64:128, 1, :],
                                   in1=dn[64:128, 1, :])

                # ---- out projection + residual (bias via PE ones-row) ----
                with tc.tile_pool(name="pF", bufs=2, space="PSUM") as pF:
                    for g in range(5):
                        real = RT_REAL[g]
                        lo = 128 * g
                        ps = pF.tile([128, 512], F32, tag="psF")
                        for q in range(4):
                            nc.tensor.matmul(
                                ps[:real],
                                oTn[:, q, lo:lo + real],
                                ow_sb[:, q, :],
                                start=(q == 0), stop=False)
                        nc.tensor.matmul(ps[:real], ones_row[0:1, :real],
                                         lnv_sb[0:1, 5, :],
                                         start=False, stop=True)
                        nc.vector.tensor_add(out=seq[:real, g, :],
                                             in0=seq[:real, g, :],
                                             in1=ps[:real])

                # ---- LN2 + MLP (with next layer's head interleaved) ----
                h2 = work.tile([128, 5, E], F16, tag="h1", name="h2", bufs=1)
                h2T = work.tile([128, 4, LRP], F16, tag="hT", name="h2T",
                                bufs=1)
                with tc.tile_pool(name="pT", bufs=2, space="PSUM") as pT:
                    for g in range(5):
                        _ln_group(h2, g, 2, 3, lnv_sb)
                        pcnt = 16 if g == 4 else 128
                        for e in range(4):
                            ptr = pT.tile([128, 128], F16, tag="ptr")
                            nc.tensor.transpose(
                                ptr[:, :pcnt],
                                h2[0:pcnt, g, 128 * e:128 * (e + 1)],
                                eye_sb[:pcnt, :pcnt])
                            nc.scalar.activation(
                                out=h2T[:, e, 128 * g:128 * g + pcnt],
                                in_=ptr[:, :pcnt], func=AF.Copy)
                with tc.tile_pool(name="pG", bufs=2, space="PSUM") as pG, \
                     tc.tile_pool(name="hp", bufs=2, space="PSUM") as hp:
                    for (rlo, rcnt) in ((0, 256), (256, 264)):
                        gT = work.tile([128, 16, 264], F16, tag="gT",
                                       name="gT", bufs=1)
                        for m in range(16):
                            ps = pG.tile([128, 512], F32, tag="psG")
                            for e in range(4):
                                nc.tensor.matmul(
                                    ps[:, :rcnt],
                                    w1_sb[:, e, 128 * m:128 * (m + 1)],
                                    h2T[:, e, rlo:rlo + rcnt],
                                    start=(e == 0), stop=(e == 3))
                            nc.scalar.activation(out=gT[:, m, :rcnt],
                                                 in_=ps[:, :rcnt],
                                                 func=AF.Gelu_apprx_tanh,
                                                 bias=pb24_sb[:, 8 + m:9 + m],
                                                 scale=1.0)
                        for (g, po, cnt, off) in _row_spans(rlo, rcnt):
                            real = min(cnt, max(0, RT_REAL[g] - po))
                            ps = pG.tile([128, 512], F32, tag="psM")
                            for cd in range(16):
                                nc.tensor.matmul(
                                    ps[:real],
                                    gT[:, cd, off:off + real],
                                    w2_sb[:, cd, :],
                                    start=(cd == 0), stop=False)
                            nc.tensor.matmul(ps[:real], ones_row[0:1, :real],
                                             lnv_sb[0:1, 6, :],
                                             start=False, stop=True)
                            nc.vector.tensor_add(out=seq[po:po + real, g, :],
                                                 in0=seq[po:po + real, g, :],
                                                 in1=ps[:real])
                            if w_next is not None:
                                head_step(w_next, hs_next, hp, g)
                w_cur = w_next
                hs_cur = hs_next

            # ================= output =================
            with tc.tile_pool(name="tail", bufs=1) as tp, \
                 tc.tile_pool(name="tailp", bufs=2, space="PSUM") as tpp:
                ngb = tp.tile([128, 2, E], F32, tag="ngb")
                nc.sync.dma_start(out=ngb[:, 0, :],
                                  in_=ng_d.partition_broadcast(128))
                nc.sync.dma_start(out=ngb[:, 1, :],
                                  in_=nb_d.partition_broadcast(128))
                hf = work.tile([128, 5, E], F16, tag="h1", name="hf2", bufs=1)
                for g in range(5):
                    cnt = 16 if g == 4 else 128
                    stats = work.tile([128, 6], F32, tag="f_st")
                    nc.vector.bn_stats(out=stats[:cnt], in_=seq[:cnt, g, :])
                    mv = work.tile([128, 2], F32, tag="f_mv")
                    nc.vector.bn_aggr(out=mv[:cnt], in_=stats[:cnt])
                    rs = work.tile([128, 1], F32, tag="f_rs")
                    _rsqrt(rs, mv, cnt)
                    y = work.tile([128, E], F32, tag="f_y")
                    nc.vector.tensor_scalar(out=y[:cnt], in0=seq[:cnt, g, :],
                                            scalar1=mv[:cnt, 0:1],
                                            scalar2=rs[:cnt],
                                            op0=mybir.AluOpType.subtract,
                                            op1=mybir.AluOpType.mult)
                    nc.vector.tensor_mul(out=y[:cnt], in0=y[:cnt],
                                         in1=ngb[:cnt, 0, :])
                    nc.vector.tensor_add(out=hf[:cnt, g, :], in0=y[:cnt],
                                         in1=ngb[:cnt, 1, :])
                hfT = tp.tile([128, 4, LRP], F16, tag="hfT", name="hfT")
                for g in range(5):
                    pcnt = 16 if g == 4 else 128
                    for e in range(4):
                        ptr = tpp.tile([128, 128], F16, tag="ptrf")
                        nc.tensor.transpose(
                            ptr[:, :pcnt],
                            hf[0:pcnt, g, 128 * e:128 * (e + 1)],
                            eye_sb[:pcnt, :pcnt])
                        nc.scalar.activation(
                            out=hfT[:, e, 128 * g:128 * g + pcnt],
                            in_=ptr[:, :pcnt], func=AF.Copy)
                hq = tp.tile([128, 4, 264], F16, tag="hq", name="hq")
                for e in range(4):
                    x = hfT[0:128, e, 32:65]
                    src = _mk_ap(x, [[_pstride(x), 128], [JJ, 8], [1, 33]])
                    nc.sync.dma_start(out=hq[:, e, :], in_=src)

                opw_sb = tp.tile([128, 4, D], F16, tag="opw")
                nc.sync.dma_start(
                    out=opw_sb, in_=opw_d.rearrange("(e p) d -> p e d", p=128))
                opb_bc = tp.tile([128, D], F16, tag="opb")
                nc.sync.dma_start(out=opb_bc, in_=opb_d.partition_broadcast(128))

                for (mlo, mcnt) in ((0, 128), (128, 128), (256, 8)):
                    ot = tp.tile([128, D], F32, tag="otile")
                    for nn in range(2):
                        ps = tpp.tile([128, 512], F32, tag="psO")
                        for e in range(4):
                            nc.tensor.matmul(
                                ps[:mcnt],
                                hq[:, e, mlo:mlo + mcnt],
                                opw_sb[:, e, 512 * nn:512 * (nn + 1)],
                                start=(e == 0), stop=(e == 3))
                        nc.vector.tensor_add(
                            out=ot[:mcnt, 512 * nn:512 * (nn + 1)],
                            in0=ps[:mcnt],
                            in1=opb_bc[:mcnt, 512 * nn:512 * (nn + 1)])
                    nc.sync.dma_start(out=out_d[mlo:mlo + mcnt, :],
                                      in_=ot[:mcnt])

    nc.compile()
    nc.m = get_hw_module(nc.m)
    return nc


# ---------------- host side ----------------

def _ln_np(x, eps=1e-5):
    m = x.mean(-1, keepdims=True)
    v = ((x - m) ** 2).mean(-1, keepdims=True)
    return (x - m) / np.sqrt(v + eps)


def _pack16(w, blocks=4):
    """[K, M] -> [128, K//128, M] with rows d = 128e + p."""
    return np.ascontiguousarray(
        w.reshape(blocks, 128, -1).transpose(1, 0, 2)).astype(np.float16)


def make_in_maps(inputs):
    f = {n: np.asarray(v) for n, v in inputs.items()}
    z = f["z_past"][0]
    code = f["code_embeddings"][0]
    q = f["query_embed"][0]
    pos = f["pos_embed"][0]

    qkw16 = np.zeros((DEPTH, 128, 4, 1024), np.float16)
    vw16 = np.zeros((DEPTH, 128, 4, 512), np.float16)
    ow16 = np.zeros((DEPTH, 128, 4, 512), np.float16)
    w1_16 = np.zeros((DEPTH, 128, 4, 2048), np.float16)
    w2_16 = np.zeros((DEPTH, 128, 16, 512), np.float16)
    pb24 = np.zeros((DEPTH, 128, 24), np.float32)
    lnvec = np.zeros((DEPTH, 7, 512), np.float16)
    for li in range(DEPTH):
        qkw16[li] = _pack16(f["qkv_w"][li][:, :1024])
        vw16[li] = _pack16(f["qkv_w"][li][:, 1024:])
        ow16[li] = _pack16(f["out_w"][li])
        w1_16[li] = _pack16(f["mlp_w1"][li])
        w2_16[li] = _pack16(f["mlp_w2"][li], blocks=16)
        pb24[li, :, :8] = f["qkv_b"][li][:1024].reshape(8, 128).T
        pb24[li, :, 8:] = f["mlp_b1"][li].reshape(16, 128).T
        lnvec[li] = np.stack([
            f["ln1_g"][li], f["ln1_b"][li], f["ln2_g"][li], f["ln2_b"][li],
            f["qkv_b"][li][1024:], f["out_b"][li], f["mlp_b2"][li]])

    # extras mask [64 keys (8c+r), 2 hh, 520 query rows]
    xm = np.zeros((64, 2, PC), np.float32)
    for c in range(T):
        for r in range(NC_):
            kx = 8 * c + r
            bad = (np.arange(PC) // JJ) < c
            if r >= 3:
                xm[kx, :, :] = -1e9
            else:
                xm[kx, :, bad] = -1e9
    shared = {
        "eye": np.eye(128, dtype=np.float16),
        "patch_w": f["patch_w"].astype(np.float16),
        "patch_b": f["patch_b"].astype(np.float32),
        "qkw16": qkw16, "vw16": vw16, "ow16": ow16,
        "w1_16": w1_16, "w2_16": w2_16,
        "pb24": pb24, "lnvec": lnvec,
        "xmask": np.ascontiguousarray(xm.reshape(64, 2 * PC)),
        "norm_g": f["norm_g"].astype(np.float32),
        "norm_b": f["norm_b"].astype(np.float32),
        "oproj_w": f["oproj_w"].astype(np.float16),
        "oproj_b": f["oproj_b"].astype(np.float16),
    }

    c_proj = _ln_np(code.astype(np.float32) @ f["code_w"] + f["code_b"])

    in_maps = []
    for k in range(NC_):
        zk = z[:, k::8, :].reshape(256, D)
        zT = np.ascontiguousarray(zk.T).astype(np.float16)
        posz = np.zeros((256, E), np.float32)
        rest = np.zeros((T * 33, E), np.float32)
        for c in range(T):
            for jj in range(32):
                posz[32 * c + jj] = pos[515 * c + 8 * jj + k]
            for jj in range(32, JJ):
                p = 8 * jj + k
                ri = 33 * c + (jj - 32)
                if p < 259:
                    rest[ri] = c_proj[c, p - 256] + pos[515 * c + p]
                elif p < CHUNK:
                    rest[ri] = q[p - 259] + pos[515 * c + p]
        m = dict(shared)
        m["zT"] = zT
        m["posz"] = posz
        m["rest"] = rest
        in_maps.append(m)
    return in_maps


def unshard_output(results, dtype):
    out = np.zeros((1, T, N, D), np.float32)
    for k in range(NC_):
        pred = results[k]["out"]
        for c in range(T):
            for i2 in range(33):
                p = 256 + 8 * i2 + k
                if 259 <= p < CHUNK:
                    out[0, c, p - 259] = pred[33 * c + i2]
    return out.astype(dtype)


_PROG_LOCK = threading.Lock()
_PROG = None


def _get_prog():
    global _PROG
    with _PROG_LOCK:
        if _PROG is None:
            _PROG = build_program()
    return _PROG


def kernel(**inputs):
    nc = _get_prog()
    in_maps = make_in_maps(inputs)
    res = run_bass_kernel_spmd(nc, in_maps, list(range(NC_)))
    return unshard_output(res.results, np.asarray(inputs["z_past"]).dtype)


if __name__ == "__main__":
    nc = build_program()
    print("program built ok")


# revision 38
# speedup vs baseline: 1.0582x; 1.0582x over previous
"""Trainium2 Bass kernel for nn_BatchedVQLAMDecoder (8-core SPMD).

Sharding: mod-8 interleave of the 4120-token sequence (8 chunks of 515,
padded to 520). Core k owns padded rows p == k (mod 8) of every chunk.
Block-causal mask at chunk granularity => uniform SPMD.

v5 (from v3):
- Cross-layer overlap: the next layer's LN1 / transposes / K,V / Q and
  the KV ship + AllGather run interleaved with the current layer's MLP
  w2 row-groups, so collectives start ~2 row-groups into the MLP.
- 2 AllGathers per layer (4 chunks each) instead of 8 small ones.
- Extras keys (the 8-wide key tail of every chunk) pulled out of the
  per-chunk inner loop into one masked per-layer pass: ~20% fewer QK/AV
  matmul passes and softmax exps.
- Software-pipelined attention inner loop: QK(i+1) issues before
  exp/AV(i) so PE keeps executing while the scalar engine runs exp.
- Transposes via PE (identity matmul) + scalar-engine copy instead of
  the serial DMA-transpose queue; QKV PSUM->SBUF moves with bias on the
  scalar engine (activation Identity); residual biases folded into the
  PSUM accumulation via a ones-row matmul.
- No rowmask multiply (padding handled by the extras mask).
"""

import threading

import numpy as np

import bass_rust
import concourse.bass as bass
import concourse.tile as tile
from concourse import bacc, mybir
from concourse.bass_utils import run_bass_kernel_spmd
from concourse.bass_interp import get_hw_module

F32 = mybir.dt.float32
F16 = mybir.dt.float16
U8 = mybir.dt.uint8
AF = mybir.ActivationFunctionType

T, N, D, CDim, E, H, DEPTH = 8, 256, 1024, 128, 512, 8, 3
Dh = E // H                   # 64
CHUNK = 2 * N + 3             # 515
JJ = 65                       # local rows per (core, chunk)
PC = 8 * JJ                   # padded chunk = 520
LR = T * JJ                   # local rows per core = 520
LRP = 528                     # padded to mult-16 for dma transpose
NC_ = 8
SCALE = 1.0 / np.sqrt(Dh)

KBYTES = E * JJ * 2           # fp16 K^T slab bytes per chunk = 66560
VW = 768                      # V slab width: 4x [V_even(64)|ones(64)|V_odd(64)]
VBYTES = JJ * VW * 2          # fp16 V slab bytes per chunk = 99840
CKVB = KBYTES + VBYTES        # 166400
PAIRB = 4 * CKVB              # AG payload: 4 chunks

RT_REAL = [128, 128, 128, 128, 8]


def _row_spans(lo, cnt):
    out = []
    done = 0
    while done < cnt:
        g = lo + done
        t = g // 128
        po = g % 128
        c = min(128 - po, cnt - done)
        out.append((t, po, c, done))
        done += c
    return out


def _mk_ap(x, dims):
    c = x.copy()
    c.ap = bass_rust.VecI64Pair([[int(a), int(b)] for a, b in dims])
    return c


def _pstride(x):
    return int([list(p) for p in x.ap][0][0])


def build_program():
    nc = bacc.Bacc("TRN2", target_bir_lowering=False, debug=False,
                   num_devices=NC_)

    def inp(name, shape, dt):
        return nc.dram_tensor(name, list(shape), dt, kind="ExternalInput").ap()

    eye_d = inp("eye", [128, 128], F16)
    zT_d = inp("zT", [D, 2 * CDim], F16)
    posz_d = inp("posz", [2 * CDim, E], F32)
    rest_d = inp("rest", [T * 33, E], F32)
    xmask_d = inp("xmask", [64, 2 * PC], F32)
    pw_d = inp("patch_w", [D, E], F16)
    pb_d = inp("patch_b", [E], F32)
    qkw_d = inp("qkw16", [DEPTH, 128, 4, 2 * E], F16)
    vw_d = inp("vw16", [DEPTH, 128, 4, E], F16)
    ow_d = inp("ow16", [DEPTH, 128, 4, E], F16)
    w1_d = inp("w1_16", [DEPTH, 128, 4, 4 * E], F16)
    w2_d = inp("w2_16", [DEPTH, 128, 16, E], F16)
    pb24_d = inp("pb24", [DEPTH, 128, 24], F32)
    lnv_d = inp("lnvec", [DEPTH, 7, E], F16)
    ng_d = inp("norm_g", [E], F32)
    nb_d = inp("norm_b", [E], F32)
    opw_d = inp("oproj_w", [E, D], F16)
    opb_d = inp("oproj_b", [D], F16)

    out_d = nc.dram_tensor("out", [T * 33, D], F32, kind="ExternalOutput").ap()

    kv_loc = [nc.dram_tensor(f"kv_loc{h}", [PAIRB], U8).ap() for h in range(2)]
    kv_g = [nc.dram_tensor(f"kv_g{h}", [NC_, PAIRB], U8,
                           addr_space="Shared").ap() for h in range(2)]

    with tile.TileContext(nc) as tc:
        import contextlib
        ctx = contextlib.ExitStack()
        with ctx:
            persist = ctx.enter_context(tc.tile_pool(name="persist", bufs=1))
            work = ctx.enter_context(tc.tile_pool(name="work", bufs=2))
            wts = ctx.enter_context(tc.tile_pool(name="wts", bufs=1))

            eps_t = persist.tile([128, 1], F32, tag="eps")
            nc.vector.memset(eps_t, 1e-5)
            xm_sb = persist.tile([64, 2, PC], F32, tag="xm")
            nc.sync.dma_start(
                out=xm_sb, in_=xmask_d.rearrange("p (h q) -> p h q", h=2))
            # V chunk tiles (2 parity): per head pair pr:
            # [V(2pr) 64 | ones 64 | V(2pr+1) 64]; 4 main key slabs only
            vc = [persist.tile([128, 4, VW], F16, tag=f"vc{par}",
                               name=f"vc{par}") for par in range(2)]
            for par in range(2):
                x = vc[par][:, 0, 64:128]
                ones_ap = _mk_ap(x, [[_pstride(x), 128], [VW, 4], [192, 4],
                                     [1, 64]])
                nc.vector.memset(ones_ap, 1.0)
            seq = persist.tile([128, 5, E], F32, tag="seq", name="seq")
            nc.vector.memset(seq, 0.0)
            eye_sb = persist.tile([128, 128], F16, tag="eye")
            nc.sync.dma_start(out=eye_sb, in_=eye_d)
            ones_row = persist.tile([1, LRP], F16, tag="ones_row")
            nc.vector.memset(ones_row, 1.0)

            def _scalar_recip(out, in_):
                inputs = [nc.scalar.lower_ap(in_)]
                for argv in (0.0, 1.0, 0.0):
                    inputs.append(mybir.ImmediateValue(
                        dtype=mybir.dt.float32, value=argv))
                return nc.scalar.add_instruction(
                    mybir.InstActivation(
                        name=nc.scalar.bass.get_next_instruction_name(),
                        func=AF.Reciprocal,
                        ins=inputs,
                        outs=[nc.scalar.lower_ap(out)]))

            def _rsqrt(rs, mv, cnt):
                nc.scalar.activation(out=rs[:cnt], in_=mv[:cnt, 1:2],
                                     func=AF.Sqrt, bias=eps_t[:cnt], scale=1.0)
                nc.vector.reciprocal(out=rs[:cnt], in_=rs[:cnt])

            # ================= layers =================
            SHIP_AFTER = {1: (0, None), 2: (2, 0), 3: (4, None), 4: (6, 1)}

            def load_weights(li):
                w = {}
                w["lnv"] = wts.tile([128, 7, E], F16, tag="lnv", bufs=2, name="lnv")
                nc.sync.dma_start(
                    out=w["lnv"],
                    in_=lnv_d[li].rearrange("v e -> (v e)")
                    .partition_broadcast(128).rearrange("p (v e) -> p v e", v=7))
                w["pb24"] = wts.tile([128, 24], F32, tag="pb24", bufs=2, name="pb24")
                nc.sync.dma_start(out=w["pb24"], in_=pb24_d[li])
                w["qkw"] = wts.tile([128, 4, 2 * E], F16, tag="qkw", name="qkw")
                nc.sync.dma_start(out=w["qkw"], in_=qkw_d[li])
                w["vw"] = wts.tile([128, 4, E], F16, tag="vw", name="vw")
                nc.sync.dma_start(out=w["vw"], in_=vw_d[li])
                w["ow"] = wts.tile([128, 4, E], F16, tag="ow", name="ow")
                nc.sync.dma_start(out=w["ow"], in_=ow_d[li])
                w["w1"] = wts.tile([128, 4, 4 * E], F16, tag="w1", name="w1")
                nc.sync.dma_start(out=w["w1"], in_=w1_d[li])
                w["w2"] = wts.tile([128, 16, E], F16, tag="w2", name="w2")
                nc.sync.dma_start(out=w["w2"], in_=w2_d[li])
                return w

            def _ship_pair(hs, c0, grp):
                for c in (c0, c0 + 1):
                    base = (c % 4) * CKVB
                    gw = c // 4
                    kdst = (kv_loc[gw][base:base + KBYTES]
                            .bitcast(F16)
                            .rearrange("(g p j) -> p g j", g=4, p=128))
                    nc.sync.dma_start(out=kdst,
                                      in_=hs["qkK"][:, :, JJ * c:JJ * c + JJ])
                    vdst = (kv_loc[gw][base + KBYTES:base + CKVB]
                            .bitcast(F16)
                            .rearrange("(l e) -> l e", e=VW))
                    for (g, po, cnt, off) in _row_spans(JJ * c, JJ):
                        nc.sync.dma_start(
                            out=vdst[off:off + cnt, :],
                            in_=hs["v16"][po:po + cnt, g, :])
                if grp is not None:
                    nc.gpsimd.collective_compute(
                        "AllGather", mybir.AluOpType.bypass,
                        replica_groups=[list(range(NC_))],
                        ins=[kv_loc[grp][:]], outs=[kv_g[grp][:, :]])

            def _ln_group(dst, g, gi, bi, lnv_sb):
                cnt = 16 if g == 4 else 128
                veng = nc.vector if g % 2 == 0 else nc.gpsimd
                stats = work.tile([128, 6], F32, tag="ln_st")
                nc.vector.bn_stats(out=stats[:cnt], in_=seq[:cnt, g, :])
                mv = work.tile([128, 2], F32, tag="ln_mv")
                nc.vector.bn_aggr(out=mv[:cnt], in_=stats[:cnt])
                rs = work.tile([128, 1], F32, tag="ln_rs")
                _rsqrt(rs, mv, cnt)
                y = work.tile([128, E], F32, tag="ln_y")
                nc.vector.tensor_scalar(out=y[:cnt], in0=seq[:cnt, g, :],
                                        scalar1=mv[:cnt, 0:1],
                                        scalar2=rs[:cnt],
                                        op0=mybir.AluOpType.subtract,
                                        op1=mybir.AluOpType.mult)
                veng.tensor_mul(out=y[:cnt], in0=y[:cnt],
                                in1=lnv_sb[:cnt, gi, :])
                veng.tensor_add(out=dst[:cnt, g, :], in0=y[:cnt],
                                in1=lnv_sb[:cnt, bi, :])

            def head_step(w, hs, hp, g):
                """LN1 group g of the next layer, its transposes (via PE),
                V(g), then the chunk-pair K/Q + ship + AllGather."""
                if g == 0:
                    hs["h16"] = work.tile([128, 5, E], F16, tag="h1",
                                          name="h1n", bufs=1)
                    hs["hT"] = work.tile([128, 4, LRP], F16, tag="hT1",
                                         name="hT1", bufs=1)
                    hs["v16"] = work.tile([128, 5, VW], F16, tag="v16",
                                          name="v16", bufs=1)
                    hs["qkQ"] = work.tile([128, 4, LRP], F16, tag="qkQ",
                                          name="qkQ", bufs=1)
                    hs["qkK"] = work.tile([128, 4, LRP], F16, tag="qkK",
                                          name="qkK", bufs=1)
                _ln_group(hs["h16"], g, 0, 1, w["lnv"])
                pcnt = 16 if g == 4 else 128
                for e in range(4):
                    ptr = hp.tile([128, 128], F16, tag="ptr")
                    nc.tensor.transpose(
                        ptr[:, :pcnt],
                        hs["h16"][0:pcnt, g, 128 * e:128 * (e + 1)],
                        eye_sb[:pcnt, :pcnt])
                    nc.scalar.activation(
                        out=hs["hT"][:, e, 128 * g:128 * g + pcnt],
                        in_=ptr[:, :pcnt], func=AF.Copy)
                real = RT_REAL[g]
                ps = hp.tile([128, 512], F32, tag="ph")
                for e in range(4):
                    nc.tensor.matmul(ps[:real],
                                     hs["hT"][:, e, 128 * g:128 * g + real],
                                     w["vw"][:, e, :],
                                     start=(e == 0), stop=(e == 3))

                def _s4(x, st, real=real):
                    return _mk_ap(x, [[_pstride(x), real], [st, 4], [1, 64]])
                v16 = hs["v16"]
                nc.vector.memset(_s4(v16[:real, g, 64:128], 192), 1.0)
                for par in range(2):
                    nc.vector.tensor_add(
                        out=_s4(v16[:real, g, 128 * par:128 * par + 64], 192),
                        in0=_s4(ps[:real, 64 * par:64 * par + 64], 128),
                        in1=_s4(w["lnv"][:real, 4, 64 * par:64 * par + 64],
                                128))
                if g in SHIP_AFTER:
                    c0, grp = SHIP_AFTER[g]
                    # K^T and Q^T columns for this chunk pair
                    for m in range(8):
                        ps2 = hp.tile([128, 512], F32, tag="ph")
                        for e in range(4):
                            nc.tensor.matmul(
                                ps2[:, :2 * JJ],
                                w["qkw"][:, e, 128 * m:128 * (m + 1)],
                                hs["hT"][:, e, JJ * c0:JJ * c0 + 2 * JJ],
                                start=(e == 0), stop=(e == 3))
                        dst = (hs["qkQ"][:, m, JJ * c0:JJ * c0 + 2 * JJ]
                               if m < 4 else
                               hs["qkK"][:, m - 4, JJ * c0:JJ * c0 + 2 * JJ])
                        nc.scalar.activation(
                            out=dst, in_=ps2[:, :2 * JJ], func=AF.Identity,
                            bias=w["pb24"][:, m:m + 1])
                    _ship_pair(hs, c0, grp)

            # ------------- front end -------------
            w_cur = load_weights(0)
            hs_cur = {}
            with tc.tile_pool(name="front", bufs=1) as fp, \
                 tc.tile_pool(name="frontp", bufs=2, space="PSUM") as fpp, \
                 tc.tile_pool(name="hp0", bufs=2, space="PSUM") as hp0:
                pb_bc = fp.tile([128, E], F32, tag="pb_bc")
                nc.sync.dma_start(out=pb_bc, in_=pb_d.partition_broadcast(128))
                zT_sb = fp.tile([128, 8, 256], F16, tag="zT")
                nc.sync.dma_start(
                    out=zT_sb, in_=zT_d.rearrange("(c p) n -> p c n", p=128))
                pw_sb = fp.tile([128, 8, E], F16, tag="pw")
                nc.sync.dma_start(
                    out=pw_sb, in_=pw_d.rearrange("(c p) n -> p c n", p=128))
                posz_sb = fp.tile([128, 2, E], F32, tag="posz")
                nc.sync.dma_start(
                    out=posz_sb, in_=posz_d.rearrange("(t p) e -> p t e", p=128))

                for zt in range(2):
                    ps = fpp.tile([128, E], F32, tag="psZ")
                    for cd in range(8):
                        nc.tensor.matmul(ps, zT_sb[:, cd, 128 * zt:128 * (zt + 1)],
                                         pw_sb[:, cd, :],
                                         start=(cd == 0), stop=(cd == 7))
                    zf = fp.tile([128, E], F32, tag="zf")
                    nc.vector.tensor_add(out=zf, in0=ps, in1=pb_bc)
                    stats = fp.tile([128, 6], F32, tag="zstats")
                    nc.vector.bn_stats(out=stats, in_=zf)
                    mv = fp.tile([128, 2], F32, tag="zmv")
                    nc.vector.bn_aggr(out=mv, in_=stats)
                    rs = fp.tile([128, 1], F32, tag="zrs")
                    _rsqrt(rs, mv, 128)
                    zn = fp.tile([128, E], F32, tag="zn")
                    nc.vector.tensor_scalar(out=zn, in0=zf,
                                            scalar1=mv[:, 0:1], scalar2=rs,
                                            op0=mybir.AluOpType.subtract,
                                            op1=mybir.AluOpType.mult)
                    nc.vector.tensor_add(out=zn, in0=zn, in1=posz_sb[:, zt, :])
                    for cb in range(4):
                        c = 4 * zt + cb
                        for (g, po, cnt, off) in _row_spans(JJ * c, 32):
                            nc.sync.dma_start(
                                out=seq[po:po + cnt, g, :],
                                in_=zn[32 * cb + off:32 * cb + off + cnt, :])
                        for (g, po, cnt, off) in _row_spans(JJ * c + 32, 33):
                            nc.sync.dma_start(
                                out=seq[po:po + cnt, g, :],
                                in_=rest_d[33 * c + off:33 * c + off + cnt, :])
                    # layer-0 head chases the assembled rows: groups 0-1
                    # (chunks 0-3) after zt=0, the rest after zt=1
                    for g in ((0, 1) if zt == 0 else (2, 3, 4)):
                        head_step(w_cur, hs_cur, hp0, g)


            for li in range(DEPTH):
                w_next = load_weights(li + 1) if li + 1 < DEPTH else None
                lnv_sb = w_cur["lnv"]
                pb24_sb = w_cur["pb24"]
                ow_sb = w_cur["ow"]
                w1_sb = w_cur["w1"]
                w2_sb = w_cur["w2"]
                qkQ = hs_cur["qkQ"]
                hs_next = {}

                # ---- attention (software-pipelined) ----
                oT = [work.tile([128, 2, PC], F32, tag=f"oT{pr}",
                                name=f"oT{pr}", bufs=1) for pr in range(4)]
                ktx = work.tile([128, 4, 64], F16, tag="ktx", bufs=1)
                vx = [work.tile([32, VW], F16, tag=f"vx{i}",
                                name=f"vx{i}", bufs=1) for i in range(2)]
                with tc.tile_pool(name="pS", bufs=2, space="PSUM") as pS, \
                     tc.tile_pool(name="pO", bufs=2, space="PSUM") as pO:
                    pend = []

                    def run_step(st):
                        (kind, pr, s, psq, poT, rlo, rcnt, cpr) = st
                        eq = work.tile([128, 2, PC], F16, tag="eq", bufs=3)
                        if kind == "main":
                            nc.scalar.activation(
                                out=eq[:, :, :rcnt], in_=psq[:, :, :rcnt],
                                func=AF.Exp, scale=float(SCALE))
                            vcp = vc[cpr % 2]
                            for hh in range(2):
                                off = 192 * pr + 64 * hh
                                nc.tensor.matmul(
                                    poT[:, hh, :rcnt],
                                    vcp[:, s, off:off + 128],
                                    eq[:, hh, :rcnt],
                                    start=(s == 0), stop=(s == 3))
                        else:
                            (k0, kn) = s
                            nc.scalar.activation(
                                out=eq[:kn, :, :rcnt], in_=psq[:kn, :, :rcnt],
                                func=AF.Exp, scale=float(SCALE))
                            for hh in range(2):
                                off = 192 * pr + 64 * hh
                                nc.tensor.matmul(
                                    poT[:, hh, :rcnt],
                                    vx[k0 // 32][:, off:off + 128],
                                    eq[:kn, hh, :rcnt],
                                    start=True, stop=True)
                        if (kind == "main" and s == 3) or kind == "x":
                            if kind == "main" and cpr == 0:
                                nc.vector.tensor_copy(
                                    out=oT[pr][:, :, rlo:rlo + rcnt],
                                    in_=poT[:, :, :rcnt])
                            else:
                                nc.vector.tensor_add(
                                    out=oT[pr][:, :, rlo:rlo + rcnt],
                                    in0=oT[pr][:, :, rlo:rlo + rcnt],
                                    in1=poT[:, :, :rcnt])

                    def push(st):
                        pend.append(st)
                        if len(pend) > 1:
                            run_step(pend.pop(0))

                    for cpr in range(T):
                        grp, rel = cpr // 4, cpr % 4
                        base = rel * CKVB
                        ktc = work.tile([128, 4, PC], F16, tag="ktc", bufs=2)
                        ksrc = (kv_g[grp][:, base:base + KBYTES]
                                .bitcast(F16)
                                .rearrange("r (g p j) -> p g r j", g=4, p=128))
                        for qb in range(4):
                            nc.sync.dma_start(
                                out=ktc[:, qb, 0:512],
                                in_=ksrc[:, qb:qb + 1, :, 0:64].squeeze(1))
                            nc.sync.dma_start(
                                out=ktc[:, qb, 512:520],
                                in_=ksrc[:, qb:qb + 1, :, 64:65]
                                .squeeze(3).squeeze(1))
                        vcp = vc[cpr % 2]
                        vsrc = (kv_g[grp][:, base + KBYTES:base + CKVB]
                                .bitcast(F16)
                                .rearrange("r (l e) -> r l e", e=VW))
                        # col kappa = 64r + l (l<64)
                        for par0 in range(2):
                            dst0 = vcp[64 * par0:64 * par0 + 64, 0, :]
                            d3 = _mk_ap(dst0, [[_pstride(dst0), 64],
                                               [VW, 4], [1, VW]])
                            nc.sync.dma_start(
                                out=d3,
                                in_=vsrc[par0:8:2, 0:64, :]
                                .rearrange("r l e -> l r e"))
                        # extras keys (l=64) of this chunk -> persistent slabs
                        nc.gpsimd.tensor_copy(
                            out=ktx[:, :, 8 * cpr:8 * cpr + 8],
                            in_=ktc[:, :, 512:520])
                        vxd = vx[cpr // 4][8 * (cpr % 4):8 * (cpr % 4) + 8, :]
                        nc.sync.dma_start(out=vxd,
                                          in_=vsrc[:, 64:65, :].squeeze(1))

                        passes = ([(0, 512), (512, 8)] if cpr == 0
                                  else [(JJ * cpr, LR - JJ * cpr)])
                        for (rlo, rcnt) in passes:
                            for pr in range(4):
                                poT = pO.tile([128, 2, 512], F32, tag="poT")
                                for s in range(4):
                                    psq = pS.tile([128, 2, 512], F32,
                                                  tag="psq")
                                    lo = 128 * s
                                    for hh in range(2):
                                        nc.tensor.matmul(
                                            psq[:, hh, :rcnt],
                                            ktc[64 * hh:64 * hh + 64,
                                                pr, lo:lo + 128],
                                            qkQ[64 * hh:64 * hh + 64,
                                                pr, rlo:rlo + rcnt],
                                            start=True, stop=True)
                                    push(("main", pr, s, psq, poT,
                                          rlo, rcnt, cpr))
                        if cpr in (3, 7):
                            # extras keys of chunks cpr-3..cpr, masked
                            k0 = 8 * (cpr - 3)
                            for (rlo, rcnt) in ((0, 512), (512, 8)):
                                for pr in range(4):
                                    poT = pO.tile([128, 2, 512], F32,
                                                  tag="poT")
                                    psq = pS.tile([128, 2, 512], F32,
                                                  tag="psq")
                                    for hh in range(2):
                                        nc.tensor.matmul(
                                            psq[:32, hh, :rcnt],
                                            ktx[64 * hh:64 * hh + 64,
                                                pr, k0:k0 + 32],
                                            qkQ[64 * hh:64 * hh + 64,
                                                pr, rlo:rlo + rcnt],
                                            start=True, stop=True)
                                    nc.vector.tensor_add(
                                        out=psq[:32, :, :rcnt],
                                        in0=psq[:32, :, :rcnt],
                                        in1=xm_sb[k0:k0 + 32, :,
                                                  rlo:rlo + rcnt])
                                    push(("x", pr, (k0, 32), psq, poT,
                                          rlo, rcnt, None))

                    while pend:
                        run_step(pend.pop(0))

                # ---- normalize (denominators already in oT) ----
                # slot0: o(2pr) @p0..64, den(2pr) @p64..128
                # slot1: den(2pr+1) @p0..64, o(2pr+1) @p64..128
                oTn = work.tile([128, 4, LRP], F16, tag="oTn", name="oTn",
                                bufs=1)
                for pr in range(4):
                    eng = nc.vector if pr % 2 == 0 else nc.gpsimd
                    dn = work.tile([128, 2, PC], F32, tag="dn", bufs=1)
                    nc.sync.dma_start(out=dn[0:64, 0, :],
                                      in_=oT[pr][64:128, 0, :])
                    nc.sync.dma_start(out=dn[64:128, 1, :],
                                      in_=oT[pr][0:64, 1, :])
                    _scalar_recip(dn[0:64, 0, :], dn[0:64, 0, :])
                    _scalar_recip(dn[64:128, 1, :], dn[64:128, 1, :])
                    eng.tensor_mul(out=oTn[0:64, pr, 0:PC],
                                   in0=oT[pr][0:64, 0, :],
                                   in1=dn[0:64, 0, :])
                    eng.tensor_mul(out=oTn[64:128, pr, 0:PC],
                                   in0=oT[pr][64:128, 1, :],
                                   in1=dn[64:128, 1, :])

                # ---- out projection + residual (bias via PE ones-row) ----
                with tc.tile_pool(name="pF", bufs=2, space="PSUM") as pF:
                    for g in range(5):
                        real = RT_REAL[g]
                        lo = 128 * g
                        ps = pF.tile([128, 512], F32, tag="psF")
                        for q in range(4):
                            nc.tensor.matmul(
                                ps[:real],
                                oTn[:, q, lo:lo + real],
                                ow_sb[:, q, :],
                                start=(q == 0), stop=False)
                        nc.tensor.matmul(ps[:real], ones_row[0:1, :real],
                                         lnv_sb[0:1, 5, :],
                                         start=False, stop=True)
                        nc.vector.tensor_add(out=seq[:real, g, :],
                                             in0=seq[:real, g, :],
                                             in1=ps[:real])

                # ---- LN2 + MLP (with next layer's head interleaved) ----
                h2 = work.tile([128, 5, E], F16, tag="h1", name="h2", bufs=1)
                h2T = work.tile([128, 4, LRP], F16, tag="hT", name="h2T",
                                bufs=1)
                with tc.tile_pool(name="pT", bufs=2, space="PSUM") as pT:
                    for g in range(5):
                        _ln_group(h2, g, 2, 3, lnv_sb)
                        pcnt = 16 if g == 4 else 128
                        for e in range(4):
                            ptr = pT.tile([128, 128], F16, tag="ptr")
                            nc.tensor.transpose(
                                ptr[:, :pcnt],
                                h2[0:pcnt, g, 128 * e:128 * (e + 1)],
                                eye_sb[:pcnt, :pcnt])
                            nc.scalar.activation(
                                out=h2T[:, e, 128 * g:128 * g + pcnt],
                                in_=ptr[:, :pcnt], func=AF.Copy)
                with tc.tile_pool(name="pG", bufs=2, space="PSUM") as pG, \
                     tc.tile_pool(name="hp", bufs=2, space="PSUM") as hp:
                    for (rlo, rcnt) in ((0, 256), (256, 264)):
                        gT = work.tile([128, 16, 264], F16, tag="gT",
                                       name="gT", bufs=1)
                        for m in range(16):
                            ps = pG.tile([128, 512], F32, tag="psG")
                            for e in range(4):
                                nc.tensor.matmul(
                                    ps[:, :rcnt],
                                    w1_sb[:, e, 128 * m:128 * (m + 1)],
                                    h2T[:, e, rlo:rlo + rcnt],
                                    start=(e == 0), stop=(e == 3))
                            nc.scalar.activation(out=gT[:, m, :rcnt],
                                                 in_=ps[:, :rcnt],
                                                 func=AF.Gelu_apprx_tanh,
                                                 bias=pb24_sb[:, 8 + m:9 + m],
                                                 scale=1.0)
                        for (g, po, cnt, off) in _row_spans(rlo, rcnt):
                            real = min(cnt, max(0, RT_REAL[g] - po))
                            ps = pG.tile([128, 512], F32, tag="psM")
                            for cd in range(16):
                                nc.tensor.matmul(
                                    ps[:real],
                                    gT[:, cd, off:off + real],
                                    w2_sb[:, cd, :],
                                    start=(cd == 0), stop=False)
                            nc.tensor.matmul(ps[:real], ones_row[0:1, :real],
                                             lnv_sb[0:1, 6, :],
                                             start=False, stop=True)
                            nc.vector.tensor_add(out=seq[po:po + real, g, :],
                                                 in0=seq[po:po + real, g, :],
                                                 in1=ps[:real])
                            if w_next is not None:
                                head_step(w_next, hs_next, hp, g)
                w_cur = w_next
                hs_cur = hs_next

            # ================= output =================
            with tc.tile_pool(name="tail", bufs=1) as tp, \
                 tc.tile_pool(name="tailp", bufs=2, space="PSUM") as tpp:
                ngb = tp.tile([128, 2, E], F32, tag="ngb")
                nc.sync.dma_start(out=ngb[:, 0, :],
                                  in_=ng_d.partition_broadcast(128))
                nc.sync.dma_start(out=ngb[:, 1, :],
                                  in_=nb_d.partition_broadcast(128))
                hf = work.tile([128, 5, E], F16, tag="h1", name="hf2", bufs=1)
                for g in range(5):
                    cnt = 16 if g == 4 else 128
                    stats = work.tile([128, 6], F32, tag="f_st")
                    nc.vector.bn_stats(out=stats[:cnt], in_=seq[:cnt, g, :])
                    mv = work.tile([128, 2], F32, tag="f_mv")
                    nc.vector.bn_aggr(out=mv[:cnt], in_=stats[:cnt])
                    rs = work.tile([128, 1], F32, tag="f_rs")
                    _rsqrt(rs, mv, cnt)
                    y = work.tile([128, E], F32, tag="f_y")
                    nc.vector.tensor_scalar(out=y[:cnt], in0=seq[:cnt, g, :],
                                            scalar1=mv[:cnt, 0:1],
                                            scalar2=rs[:cnt],
                                            op0=mybir.AluOpType.subtract,
                                            op1=mybir.AluOpType.mult)
                    nc.vector.tensor_mul(out=y[:cnt], in0=y[:cnt],
                                         in1=ngb[:cnt, 0, :])
                    nc.vector.tensor_add(out=hf[:cnt, g, :], in0=y[:cnt],
                                         in1=ngb[:cnt, 1, :])
                hfT = tp.tile([128, 4, LRP], F16, tag="hfT", name="hfT")
                for g in range(5):
                    pcnt = 16 if g == 4 else 128
                    for e in range(4):
                        ptr = tpp.tile([128, 128], F16, tag="ptrf")
                        nc.tensor.transpose(
                            ptr[:, :pcnt],
                            hf[0:pcnt, g, 128 * e:128 * (e + 1)],
                            eye_sb[:pcnt, :pcnt])
                        nc.scalar.activation(
                            out=hfT[:, e, 128 * g:128 * g + pcnt],
                            in_=ptr[:, :pcnt], func=AF.Copy)
                hq = tp.tile([128, 4, 264], F16, tag="hq", name="hq")
                for e in range(4):
                    x = hfT[0:128, e, 32:65]
                    src = _mk_ap(x, [[_pstride(x), 128], [JJ, 8], [1, 33]])
                    nc.sync.dma_start(out=hq[:, e, :], in_=src)

                opw_sb = tp.tile([128, 4, D], F16, tag="opw")
                nc.sync.dma_start(
                    out=opw_sb, in_=opw_d.rearrange("(e p) d -> p e d", p=128))
                opb_bc = tp.tile([128, D], F16, tag="opb")
                nc.sync.dma_start(out=opb_bc, in_=opb_d.partition_broadcast(128))

                for (mlo, mcnt) in ((0, 128), (128, 128), (256, 8)):
                    ot = tp.tile([128, D], F32, tag="otile")
                    for nn in range(2):
                        ps = tpp.tile([128, 512], F32, tag="psO")
                        for e in range(4):
                            nc.tensor.matmul(
                                ps[:mcnt],
                                hq[:, e, mlo:mlo + mcnt],
                                opw_sb[:, e, 512 * nn:512 * (nn + 1)],
                                start=(e == 0), stop=(e == 3))
                        nc.vector.tensor_add(
                            out=ot[:mcnt, 512 * nn:512 * (nn + 1)],
                            in0=ps[:mcnt],
                            in1=opb_bc[:mcnt, 512 * nn:512 * (nn + 1)])
                    nc.sync.dma_start(out=out_d[mlo:mlo + mcnt, :],
                                      in_=ot[:mcnt])

    nc.compile()
    nc.m = get_hw_module(nc.m)
    return nc


# ---------------- host side ----------------

def _ln_np(x, eps=1e-5):
    m = x.mean(-1, keepdims=True)
    v = ((x - m) ** 2).mean(-1, keepdims=True)
    return (x - m) / np.sqrt(v + eps)


def _pack16(w, blocks=4):
    """[K, M] -> [128, K//128, M] with rows d = 128e + p."""
    return np.ascontiguousarray(
        w.reshape(blocks, 128, -1).transpose(1, 0, 2)).astype(np.float16)


def make_in_maps(inputs):
    f = {n: np.asarray(v) for n, v in inputs.items()}
    z = f["z_past"][0]
    code = f["code_embeddings"][0]
    q = f["query_embed"][0]
    pos = f["pos_embed"][0]

    qkw16 = np.zeros((DEPTH, 128, 4, 1024), np.float16)
    vw16 = np.zeros((DEPTH, 128, 4, 512), np.float16)
    ow16 = np.zeros((DEPTH, 128, 4, 512), np.float16)
    w1_16 = np.zeros((DEPTH, 128, 4, 2048), np.float16)
    w2_16 = np.zeros((DEPTH, 128, 16, 512), np.float16)
    pb24 = np.zeros((DEPTH, 128, 24), np.float32)
    lnvec = np.zeros((DEPTH, 7, 512), np.float16)
    for li in range(DEPTH):
        qkw16[li] = _pack16(f["qkv_w"][li][:, :1024])
        vw16[li] = _pack16(f["qkv_w"][li][:, 1024:])
        ow16[li] = _pack16(f["out_w"][li])
        w1_16[li] = _pack16(f["mlp_w1"][li])
        w2_16[li] = _pack16(f["mlp_w2"][li], blocks=16)
        pb24[li, :, :8] = f["qkv_b"][li][:1024].reshape(8, 128).T
        pb24[li, :, 8:] = f["mlp_b1"][li].reshape(16, 128).T
        lnvec[li] = np.stack([
            f["ln1_g"][li], f["ln1_b"][li], f["ln2_g"][li], f["ln2_b"][li],
            f["qkv_b"][li][1024:], f["out_b"][li], f["mlp_b2"][li]])

    # extras mask [64 keys (8c+r), 2 hh, 520 query rows]
    xm = np.zeros((64, 2, PC), np.float32)
    for c in range(T):
        for r in range(NC_):
            kx = 8 * c + r
            bad = (np.arange(PC) // JJ) < c
            if r >= 3:
                xm[kx, :, :] = -1e9
            else:
                xm[kx, :, bad] = -1e9
    shared = {
        "eye": np.eye(128, dtype=np.float16),
        "patch_w": f["patch_w"].astype(np.float16),
        "patch_b": f["patch_b"].astype(np.float32),
        "qkw16": qkw16, "vw16": vw16, "ow16": ow16,
        "w1_16": w1_16, "w2_16": w2_16,
        "pb24": pb24, "lnvec": lnvec,
        "xmask": np.ascontiguousarray(xm.reshape(64, 2 * PC)),
        "norm_g": f["norm_g"].astype(np.float32),
        "norm_b": f["norm_b"].astype(np.float32),
        "oproj_w": f["oproj_w"].astype(np.float16),
        "oproj_b": f["oproj_b"].astype(np.float16),
    }

    c_proj = _ln_np(code.astype(np.float32) @ f["code_w"] + f["code_b"])

    in_maps = []
    for k in range(NC_):
        zk = z[:, k::8, :].reshape(256, D)
        zT = np.ascontiguousarray(zk.T).astype(np.float16)
        posz = np.zeros((256, E), np.float32)
        rest = np.zeros((T * 33, E), np.float32)
        for c in range(T):
            for jj in range(32):
                posz[32 * c + jj] = pos[515 * c + 8 * jj + k]
            for jj in range(32, JJ):
                p = 8 * jj + k
                ri = 33 * c + (jj - 32)
                if p < 259:
                    rest[ri] = c_proj[c, p - 256] + pos[515 * c + p]
                elif p < CHUNK:
                    rest[ri] = q[p - 259] + pos[515 * c + p]
        m = dict(shared)
        m["zT"] = zT
        m["posz"] = posz
        m["rest"] = rest
        in_maps.append(m)
    return in_maps


def unshard_output(results, dtype):
    out = np.zeros((1, T, N, D), np.float32)
    for k in range(NC_):
        pred = results[k]["out"]
        for c in range(T):
            for i2 in range(33):
                p = 256 + 8 * i2 + k
                if 259 <= p < CHUNK:
                    out[0, c, p - 259] = pred[33 * c + i2]
    return out.astype(dtype)


_PROG_LOCK = threading.Lock()
_PROG = None


def _get_prog():
    global _PROG
    with _PROG_LOCK:
        if _PROG is None:
            _PROG = build_program()
    return _PROG


def kernel(**inputs):
    nc = _get_prog()
    in_maps = make_in_maps(inputs)
    res = run_bass_kernel_spmd(nc, in_maps, list(range(NC_)))
    return unshard_output(res.results, np.asarray(inputs["z_past"]).dtype)


if __name__ == "__main__":
    nc = build_program()
    print("program built ok")
